# revision 1
# baseline (speedup 1.0000x reference)
"""Sparse (sliding-window) attention head on 8 TRN2 NeuronCores.

Reference computation (B=2, S=4096, D=512, HD=64, SCALE=128):
    q = x @ wq ; k = x @ wk ; v = x @ wv          [B,S,64]
    scores[b,s,w] = q[b,s] . k[b,s-128+w] / 8     w in [0,256), zero-padded OOB
    out = softmax_w(scores) @ v_window            [B,S,64]

Sharding: 8 shards = (batch b, 1024-seq chunk c). Each shard gets a
zero-padded 128-halo of x on both sides, which reproduces the reference's
zero-padded (not masked) window semantics exactly. All compute is local,
no collectives.

Device layout (per core):
    xT   [512,1280] bf16  host-pre-transposed padded input shard
    w3   [128,3,4,64] bf16  packed wq|wk|wv (d-chunk on partitions)
    mask [128,384] bf16  band-validity mask for one 128-query block
    out  [1024,64] f32

    qT,kT = w.T @ xT   (head dim on partitions)
    v     = xT.T @ wv  (natural layout, keys on partitions) + ones column
    per 128-query block qb:
        scT[key,que] = kT_chunk.T @ qT_block      3 chunks of [128,128]
        e = exp(scT/8) * mask                     bf16
        av[que,0:65] = sum_c e_c.T @ vaug_c       ones col -> softmax denom
        out_block = av[:, :64] * (1/av[:, 64])
"""

import sys
import types

import numpy as np
import ml_dtypes

B, S, D = 2, 4096, 512
HD = 64
SCALE = 128
SS = S // 4          # 1024 positions per shard
HP = SCALE           # halo padding each side
NP = SS + 2 * HP     # 1280 padded positions
NKC = NP // 128      # 10 key chunks
NQB = SS // 128      # 8 query blocks
NDC = D // 128       # 4 d-chunks

_CACHE = {}


def _ensure_hooks():
    """Register the axon NTFF profile hook; keep artifacts local."""
    if "antenv.axon_hooks" not in sys.modules:
        try:
            from trn_agent_boot.trn_boot import _ntff_profile_via_ctypes

            m = types.ModuleType("antenv.axon_hooks")
            m.get_axon_ntff_profile_hook = lambda: _ntff_profile_via_ctypes(
                "/opt/axon/libaxon_pjrt.so"
            )
            sys.modules["antenv.axon_hooks"] = m
        except Exception:
            pass
    import concourse.bass_utils as bass_utils

    bass_utils.upload_artifacts = lambda tmpdir: tmpdir


def _build_nc():
    import concourse.mybir as mybir
    import concourse.tile as tile
    from concourse import bacc

    bf = mybir.dt.bfloat16
    f32 = mybir.dt.float32
    AF = mybir.ActivationFunctionType

    nc = bacc.Bacc("TRN2", target_bir_lowering=False, debug=False, num_devices=8)

    xT_d = nc.dram_tensor("xT", [D, NP], bf, kind="ExternalInput")
    w3_d = nc.dram_tensor("w3", [128, 3, NDC, HD], bf, kind="ExternalInput")
    mask_d = nc.dram_tensor("mask", [128, 384], bf, kind="ExternalInput")
    out_d = nc.dram_tensor("out", [SS, HD], f32, kind="ExternalOutput")

    with tile.TileContext(nc) as tc:
        with (
            tc.tile_pool(name="consts", bufs=1) as consts,
            tc.tile_pool(name="xtp", bufs=1) as xtp,
            tc.tile_pool(name="qkp", bufs=1) as qkp,
            tc.tile_pool(name="vgp", bufs=1) as vgp,
            tc.tile_pool(name="work", bufs=3) as work,
            tc.tile_pool(name="fin", bufs=3) as fin,
        ):
            w_s = consts.tile([128, 3, NDC, HD], bf)
            nc.sync.dma_start(out=w_s, in_=w3_d[:, :, :, :])
            mask_s = consts.tile([128, 384], bf)
            nc.sync.dma_start(out=mask_s, in_=mask_d[:, :])

            # Trigger the ACT exp table load early so it hides under DMA/proj.
            zz = consts.tile([128, 1], f32)
            nc.vector.memset(zz, 0.0)
            ez = consts.tile([128, 1], f32)
            nc.scalar.activation(ez, zz, AF.Exp)

            # Few big DMAs: each HWDGE dma_start costs ~600ns of serial
            # issue time on the Sync sequencer.
            xt = []
            for dc in range(NDC):
                t = xtp.tile([128, NP], bf, tag=f"xt{dc}")
                nc.sync.dma_start(out=t, in_=xT_d[dc * 128 : (dc + 1) * 128, :])
                xt.append(t)

            # DMA-free garbage tile for PE warmup.
            garb = consts.tile([128, 512], bf)
            nc.vector.memset(garb, 0.5)

            qT_s = qkp.tile([64, SS], bf, tag="qT")
            kT_s = qkp.tile([64, NP], bf, tag="kT")
            vaug = vgp.tile([128, NKC, 66], bf)
            nc.vector.memset(vaug[:, :, 64:66], 1.0)

            with (
                tc.tile_pool(name="wrm", bufs=1, space="PSUM") as wrm,
                tc.tile_pool(name="pps", bufs=3, space="PSUM") as pps,
                tc.tile_pool(name="vps", bufs=3, space="PSUM") as vps,
            ):
                # PE warmup: dummy matmuls on the memset tile while the xT
                # DMAs land, so HAM un-throttles (1.2 -> 2.4 GHz) before the
                # real work reaches the array.
                wps = wrm.tile([64, 512], f32, tag="warm")
                for _ in range(7):
                    nc.tensor.matmul(
                        wps,
                        lhsT=garb[:, 0:64],
                        rhs=garb[:, :],
                        start=True,
                        stop=True,
                    )

                # qT / kT projections: head dim on partitions. kT evacs on
                # ACT, qT on DVE to split the PSUM-drain load.
                segs_q = [(0, 512), (512, 512)]
                segs_k = [(0, 512), (512, 512), (1024, 256)]
                for j, dst, off, segs in ((1, kT_s, 0, segs_k), (0, qT_s, HP, segs_q)):
                    for s0, w in segs:
                        ps = pps.tile([64, 512], f32, tag="pp")
                        for dc in range(NDC):
                            nc.tensor.matmul(
                                ps[:, :w],
                                lhsT=w_s[:, j, dc, :],
                                rhs=xt[dc][:, off + s0 : off + s0 + w],
                                start=(dc == 0),
                                stop=(dc == NDC - 1),
                            )
                        if j == 1:
                            nc.scalar.copy(dst[:, s0 : s0 + w], ps[:, :w])
                        else:
                            nc.vector.tensor_copy(dst[:, s0 : s0 + w], ps[:, :w])

                # v in natural layout (keys on partitions) + ones column.
                for kc in range(NKC):
                    vp = vps.tile([128, HD], f32, tag="vp")
                    for dc in range(NDC):
                        nc.tensor.matmul(
                            vp,
                            lhsT=xt[dc][:, kc * 128 : (kc + 1) * 128],
                            rhs=w_s[:, 2, dc, :],
                            start=(dc == 0),
                            stop=(dc == NDC - 1),
                        )
                    nc.vector.tensor_copy(vaug[:, kc, 0:64], vp)

            with (
                tc.tile_pool(name="sps", bufs=3, space="PSUM") as sps,
                tc.tile_pool(name="aps", bufs=3, space="PSUM") as aps,
            ):
                for qb in range(NQB):
                    sc = sps.tile([128, 384], f32, tag="sc")
                    for c in range(3):
                        nc.tensor.matmul(
                            sc[:, c * 128 : (c + 1) * 128],
                            lhsT=kT_s[:, (qb + c) * 128 : (qb + c + 1) * 128],
                            rhs=qT_s[:, qb * 128 : (qb + 1) * 128],
                            start=True,
                            stop=True,
                        )
                    ex = work.tile([128, 384], bf, tag="ex")
                    nc.scalar.activation(ex, sc, AF.Exp, scale=0.125)
                    em = work.tile([128, 384], bf, tag="em")
                    nc.vector.tensor_mul(em, ex, mask_s)
                    av = aps.tile([128, 65], f32, tag="av")
                    for c in range(3):
                        nc.tensor.matmul(
                            av,
                            lhsT=em[:, c * 128 : (c + 1) * 128],
                            rhs=vaug[:, qb + c, 0:65],
                            start=(c == 0),
                            stop=(c == 2),
                        )
                    rc = fin.tile([128, 1], f32, tag="rc")
                    nc.vector.reciprocal(rc, av[:, 64:65])
                    ot = fin.tile([128, HD], f32, tag="ot")
                    nc.vector.tensor_scalar_mul(ot, av[:, 0:HD], rc)
                    nc.sync.dma_start(
                        out=out_d[qb * 128 : (qb + 1) * 128, :], in_=ot
                    )

    nc.compile()
    return nc


def _get_nc():
    if "nc" not in _CACHE:
        _ensure_hooks()
        _CACHE["nc"] = _build_nc()
    return _CACHE["nc"]


def _host_inputs(inputs, wq, wk, wv):
    bf16 = ml_dtypes.bfloat16
    x = np.asarray(inputs, dtype=np.float32)

    # w3[p, j, c, m] = w_j[c*128 + p, m]
    w3 = np.stack([wq, wk, wv]).astype(np.float32)          # [3, 512, 64]
    w3 = w3.reshape(3, NDC, 128, HD).transpose(2, 0, 1, 3)   # [128, 3, 4, 64]
    w3 = np.ascontiguousarray(w3).astype(bf16)

    p = np.arange(128)[:, None]
    q = np.arange(128)[None, :]
    mask = np.concatenate(
        [(p >= q), np.ones((128, 128), bool), (p < q)], axis=1
    ).astype(bf16)                                           # [128, 384]

    in_maps = []
    for i in range(8):
        b, c = divmod(i, 4)
        s0 = c * SS
        xp = np.zeros((NP, D), np.float32)
        lo = max(0, s0 - HP)
        hi = min(S, s0 + SS + HP)
        xp[lo - (s0 - HP) : hi - (s0 - HP)] = x[b, lo:hi]
        xT = np.ascontiguousarray(xp.T).astype(bf16)         # [512, 1280]
        in_maps.append({"xT": xT, "w3": w3, "mask": mask})
    return in_maps


def run_sharded(inputs, wq, wk, wv, trace=False, trace_cores=None):
    """Run the SPMD kernel; returns (out [B,S,HD] f32, BassKernelResults)."""
    _ensure_hooks()
    import concourse.bass_utils as bass_utils

    nc = _get_nc()
    in_maps = _host_inputs(inputs, wq, wk, wv)
    res = bass_utils.run_bass_kernel_spmd(
        nc,
        in_maps,
        core_ids=list(range(8)),
        trace=trace,
        trace_cores=trace_cores,
    )
    out = np.empty((B, S, HD), np.float32)
    for i in range(8):
        b, c = divmod(i, 4)
        out[b, c * SS : (c + 1) * SS] = res.results[i]["out"]
    return out, res


def kernel(inputs, wq, wk, wv):
    out, _ = run_sharded(inputs, wq, wk, wv, trace=False)
    return out



# revision 3
# speedup vs baseline: 1.1366x; 1.1366x over previous
"""Sparse (sliding-window) attention head on 8 TRN2 NeuronCores.

Reference computation (B=2, S=4096, D=512, HD=64, SCALE=128):
    q = x @ wq ; k = x @ wk ; v = x @ wv          [B,S,64]
    scores[b,s,w] = q[b,s] . k[b,s-128+w] / 8     w in [0,256), zero-padded OOB
    out = softmax_w(scores) @ v_window            [B,S,64]

Sharding: 8 shards = (batch b, 1024-seq chunk c). Each shard gets a
zero-padded 128-halo of x on both sides, which reproduces the reference's
zero-padded (not masked) window semantics exactly. All compute is local,
no collectives.

v2 schedule (vs the v1 baseline):
  - input DMAs split across both HWDGE engines (sync + scalar), issued
    before anything else; ACT exp-table preload emitted after the issues
  - continuous PE warmup matmuls under the DMA window keep the HAM
    activity monitor from holding the PE at 1.2 GHz
  - q and k projected in ONE packed matmul group (lhsT = [wq|wk], M=128)
    accumulated across d-chunks in a single 3-bank PSUM tile; evacuated
    as partition-shifted copies (q rows 0:64, k rows 64:128)
  - v projected keys-on-partitions, kc-outer, evacuated in 3 batched
    copies instead of 10
  - attention processed in 2 groups of 4 query blocks: 12 score matmuls
    into one 3-bank PSUM tile, ONE big exp over [128,4,3,128], one
    masked multiply for the two triangular side-chunks only (middle
    chunk of the band is always fully valid)
  - softmax denominator via ones-column in the AV matmul; final divide
    batched: one reciprocal + one broadcast multiply per group
  - output written as ONE DMA of [128, 8, 64] bf16
"""

import sys
import types

import numpy as np
import ml_dtypes

B, S, D = 2, 4096, 512
HD = 64
SCALE = 128
SS = S // 4          # 1024 positions per shard
HP = SCALE           # halo padding each side
NP = SS + 2 * HP     # 1280 padded positions
NKC = NP // 128      # 10 key chunks
NQB = SS // 128      # 8 query blocks
NDC = D // 128       # 4 d-chunks

WARMUP = 8           # PE warmup matmuls (~3.4us at 1.2GHz covers the HAM window)

_CACHE = {}


def _ensure_hooks():
    """Register the axon NTFF profile hook; keep artifacts local."""
    if "antenv.axon_hooks" not in sys.modules:
        try:
            from trn_agent_boot.trn_boot import _ntff_profile_via_ctypes

            m = types.ModuleType("antenv.axon_hooks")
            m.get_axon_ntff_profile_hook = lambda: _ntff_profile_via_ctypes(
                "/opt/axon/libaxon_pjrt.so"
            )
            sys.modules["antenv.axon_hooks"] = m
        except Exception:
            pass
    import concourse.bass_utils as bass_utils

    bass_utils.upload_artifacts = lambda tmpdir: tmpdir


def _build_nc():
    import concourse.mybir as mybir
    import concourse.tile as tile
    from concourse import bacc

    bf = mybir.dt.bfloat16
    f32 = mybir.dt.float32
    AF = mybir.ActivationFunctionType
    AL = mybir.AluOpType

    nc = bacc.Bacc("TRN2", target_bir_lowering=False, debug=False, num_devices=8)

    xT_d = nc.dram_tensor("xT", [D, NP], bf, kind="ExternalInput")
    wm_d = nc.dram_tensor("wm", [128, 8, 128], bf, kind="ExternalInput")
    out_d = nc.dram_tensor("out", [128, NQB, HD], bf, kind="ExternalOutput")

    with tile.TileContext(nc) as tc:
        with (
            tc.tile_pool(name="consts", bufs=1) as consts,
            tc.tile_pool(name="xtp", bufs=1) as xtp,
            tc.tile_pool(name="qkp", bufs=1) as qkp,
            tc.tile_pool(name="vgp", bufs=1) as vgp,
            tc.tile_pool(name="exp", bufs=2) as expool,
            tc.tile_pool(name="emp", bufs=2) as empool,
            tc.tile_pool(name="fin", bufs=1) as fin,
        ):
            wm_s = consts.tile([128, 8, 128], bf)
            xt = [xtp.tile([128, NP], bf, tag=f"xt{dc}", name=f"xt{dc}") for dc in range(NDC)]

            # Input DMAs first, split across both HWDGE engines so the
            # issue serialization (~0.7us each) overlaps.
            nc.scalar.dma_start(out=wm_s, in_=wm_d[:, :, :])
            nc.sync.dma_start(out=xt[0], in_=xT_d[0:128, :])
            nc.scalar.dma_start(out=xt[1], in_=xT_d[128:256, :])
            nc.sync.dma_start(out=xt[2], in_=xT_d[256:384, :])
            nc.scalar.dma_start(out=xt[3], in_=xT_d[384:512, :])

            # Trigger the ACT exp table load now (1.3us), after the DMA
            # issues, so it hides under the transfers.
            zz = consts.tile([128, 1], f32)
            nc.vector.memset(zz, 0.0)
            ez = consts.tile([128, 1], f32)
            nc.scalar.activation(ez, zz, AF.Exp)

            # DMA-free garbage tile for PE warmup.
            garb = consts.tile([128, 512], bf)
            nc.vector.memset(garb, 0.5)

            kT = qkp.tile([64, NP], bf, tag="kT")
            qT = qkp.tile([64, SS], bf, tag="qT")
            vaug = vgp.tile([128, NKC, 66], bf)
            nc.vector.memset(vaug[:, :, 64:66], 1.0)

            outsb = fin.tile([128, NQB, HD], bf)
            rc = fin.tile([128, NQB, 1], f32)

            with (
                tc.tile_pool(name="wrm", bufs=1, space="PSUM") as wrm,
                tc.tile_pool(name="pps", bufs=1, space="PSUM") as pps,
                tc.tile_pool(name="vps", bufs=1, space="PSUM") as vps,
            ):
                # PE warmup: continuous dummy matmuls while the xT DMAs
                # land, so HAM un-throttles (1.2 -> 2.4 GHz) before the
                # real work reaches the array.
                wps = wrm.tile([128, 512], f32, tag="warm")
                for _ in range(WARMUP):
                    nc.tensor.matmul(
                        wps, lhsT=garb[:, 0:128], rhs=garb, start=True, stop=True
                    )

                # Packed q|k projection: lhsT = [wq|wk] per d-chunk,
                # M=128, accumulated over d-chunks into one 3-bank tile.
                # d-chunk outer so matmuls start as each xt DMA lands.
                qks = pps.tile([128, 3, 512], f32, tag="qk")
                for dc in range(NDC):
                    for si in range(3):
                        w = 512 if si < 2 else 256
                        nc.tensor.matmul(
                            qks[:, si, 0:w],
                            lhsT=wm_s[:, dc, :],
                            rhs=xt[dc][:, si * 512 : si * 512 + w],
                            start=(dc == 0),
                            stop=(dc == NDC - 1),
                        )
                # Evacuate: k rows 64:128 -> kT (all 1280 cols) on ACT,
                # q rows 0:64 -> qT (only the 1024 real queries) on DVE.
                nc.scalar.copy(kT[:, 0:512], qks[64:128, 0, :])
                nc.scalar.copy(kT[:, 512:1024], qks[64:128, 1, :])
                nc.scalar.copy(kT[:, 1024:1280], qks[64:128, 2, 0:256])
                nc.vector.tensor_copy(qT[:, 0:384], qks[0:64, 0, 128:512])
                nc.vector.tensor_copy(qT[:, 384:896], qks[0:64, 1, :])
                nc.vector.tensor_copy(qT[:, 896:1024], qks[0:64, 2, 0:128])

                # v in natural layout (keys on partitions), kc-outer so
                # each 4-chunk PSUM bank finishes early; 3 batched evacs.
                vt = [vps.tile([128, 4, HD], f32, tag=f"v{g}", name=f"v{g}") for g in range(3)]
                for kc in range(NKC):
                    for dc in range(NDC):
                        nc.tensor.matmul(
                            vt[kc // 4][:, kc % 4, :],
                            lhsT=xt[dc][:, kc * 128 : (kc + 1) * 128],
                            rhs=wm_s[:, 4 + dc // 2, (dc % 2) * 64 : (dc % 2) * 64 + 64],
                            start=(dc == 0),
                            stop=(dc == NDC - 1),
                        )
                    if kc == 3:
                        nc.vector.tensor_copy(vaug[:, 0:4, 0:64], vt[0])
                    elif kc == 7:
                        nc.vector.tensor_copy(vaug[:, 4:8, 0:64], vt[1])
                    elif kc == 9:
                        nc.vector.tensor_copy(vaug[:, 8:10, 0:64], vt[2][:, 0:2, :])

            with (
                tc.tile_pool(name="sps", bufs=2, space="PSUM") as sps,
                tc.tile_pool(name="aps", bufs=1, space="PSUM") as aps,
            ):
                mask_b = wm_s[:, 6:8, :].unsqueeze(1).broadcast_to([128, 4, 2, 128])
                avs = []
                # Scores for both groups first (PE side), AV after — the
                # bufs=2 sc pool lets group 1 scores run while group 0's
                # exp/mask are on ACT/DVE.
                exs, ems = [], []
                for g in range(2):
                    sc4 = sps.tile([128, 4, 3, 128], f32, tag="sc")
                    for b in range(4):
                        qb = g * 4 + b
                        for c in range(3):
                            nc.tensor.matmul(
                                sc4[:, b, c, :],
                                lhsT=kT[:, (qb + c) * 128 : (qb + c + 1) * 128],
                                rhs=qT[:, qb * 128 : (qb + 1) * 128],
                                start=True,
                                stop=True,
                            )
                    ex4 = expool.tile([128, 4, 3, 128], bf, tag="ex")
                    nc.scalar.activation(ex4, sc4, AF.Exp, scale=0.125)
                    em4 = empool.tile([128, 4, 2, 128], bf, tag="em")
                    nc.vector.tensor_tensor(
                        out=em4, in0=ex4[:, :, 0:3:2, :], in1=mask_b, op=AL.mult
                    )
                    av = aps.tile([128, 4, 65], f32, tag=f"av{g}", name=f"av{g}")
                    for b in range(4):
                        qb = g * 4 + b
                        for c in range(3):
                            lhsT = (
                                em4[:, b, 0, :]
                                if c == 0
                                else (ex4[:, b, 1, :] if c == 1 else em4[:, b, 1, :])
                            )
                            nc.tensor.matmul(
                                av[:, b, 0:65],
                                lhsT=lhsT,
                                rhs=vaug[:, qb + c, 0:65],
                                start=(c == 0),
                                stop=(c == 2),
                            )
                    avs.append(av)

                for g in range(2):
                    av = avs[g]
                    sl = slice(g * 4, (g + 1) * 4)
                    nc.vector.reciprocal(rc[:, sl, :], av[:, :, 64:65])
                    nc.vector.tensor_tensor(
                        out=outsb[:, sl, :],
                        in0=av[:, :, 0:64],
                        in1=rc[:, sl, :].broadcast_to([128, 4, 64]),
                        op=AL.mult,
                    )
                nc.sync.dma_start(out=out_d[:, :, :], in_=outsb)

    nc.compile()
    return nc


def _get_nc():
    if "nc" not in _CACHE:
        _ensure_hooks()
        _CACHE["nc"] = _build_nc()
    return _CACHE["nc"]


def _host_inputs(inputs, wq, wk, wv):
    bf16 = ml_dtypes.bfloat16
    x = np.asarray(inputs, dtype=np.float32)
    wq = np.asarray(wq, dtype=np.float32)
    wk = np.asarray(wk, dtype=np.float32)
    wv = np.asarray(wv, dtype=np.float32)

    wm = np.zeros((128, 8, 128), np.float32)
    for dc in range(NDC):
        wm[:, dc, 0:64] = wq[dc * 128 : (dc + 1) * 128, :]
        wm[:, dc, 64:128] = wk[dc * 128 : (dc + 1) * 128, :]
    for dc in range(NDC):
        wm[:, 4 + dc // 2, (dc % 2) * 64 : (dc % 2) * 64 + 64] = (
            wv[dc * 128 : (dc + 1) * 128, :]
        )
    p = np.arange(128)[:, None]
    q = np.arange(128)[None, :]
    wm[:, 6, :] = (p >= q).astype(np.float32)
    wm[:, 7, :] = (p < q).astype(np.float32)
    wm = wm.astype(bf16)

    in_maps = []
    for i in range(8):
        b, c = divmod(i, 4)
        s0 = c * SS
        xp = np.zeros((NP, D), np.float32)
        lo = max(0, s0 - HP)
        hi = min(S, s0 + SS + HP)
        xp[lo - (s0 - HP) : hi - (s0 - HP)] = x[b, lo:hi]
        xT = np.ascontiguousarray(xp.T).astype(bf16)         # [512, 1280]
        in_maps.append({"xT": xT, "wm": wm})
    return in_maps


def run_sharded(inputs, wq, wk, wv, trace=False, trace_cores=None):
    """Run the SPMD kernel; returns (out [B,S,HD] f32, BassKernelResults)."""
    _ensure_hooks()
    import concourse.bass_utils as bass_utils

    nc = _get_nc()
    in_maps = _host_inputs(inputs, wq, wk, wv)
    res = bass_utils.run_bass_kernel_spmd(
        nc,
        in_maps,
        core_ids=list(range(8)),
        trace=trace,
        trace_cores=trace_cores,
    )
    out = np.empty((B, S, HD), np.float32)
    for i in range(8):
        b, c = divmod(i, 4)
        o = np.asarray(res.results[i]["out"]).astype(np.float32)  # [128, 8, 64]
        out[b, c * SS : (c + 1) * SS] = o.transpose(1, 0, 2).reshape(SS, HD)
    return out, res


def kernel(inputs, wq, wk, wv):
    out, _ = run_sharded(inputs, wq, wk, wv, trace=False)
    return out


# revision 7
# speedup vs baseline: 1.3282x; 1.1686x over previous
"""Sparse (sliding-window) attention head on 8 TRN2 NeuronCores.

Reference computation (B=2, S=4096, D=512, HD=64, SCALE=128):
    q = x @ wq ; k = x @ wk ; v = x @ wv          [B,S,64]
    scores[b,s,w] = q[b,s] . k[b,s-128+w] / 8     w in [0,256), zero-padded OOB
    out = softmax_w(scores) @ v_window            [B,S,64]

Sharding: 8 shards = (batch b, 1024-seq chunk c). Each shard gets a
zero-padded 128-halo of x on both sides, which reproduces the reference's
zero-padded (not masked) window semantics exactly. All compute is local,
no collectives.

v3 schedule — the kernel is input-DMA-bound (~1.6MB over ~240GB/s shared
by 16 DMA engines), so the shard is processed in two sequence REGIONS to
overlap region-A attention with region-B's input transfer:
  region A = padded cols 0:768  -> query blocks 0-3 (self-contained)
  region B = padded cols 768:1280 -> query blocks 4-7 (keys 512:1280)
  - 9 input DMAs balanced across both HWDGE engines (sync+scalar)
  - packed [wq|wk] projection per region (M=128), PSUM-accumulated over
    d-chunks; q/k evac of a region stays on ONE engine (ACT for A, DVE
    for B) because ACT+DVE cannot read the same PSUM banks in parallel
  - v natural-layout, interleaved with qk per d-chunk, batched evacs
  - attention in 2 groups of 4 blocks: 12 score matmuls into a 3-bank
    PSUM tile, one big exp, one masked multiply of the triangular side
    chunks only; AV matmuls write back INTO the score PSUM tile (frees
    2 banks) with the softmax denominator via a ones-column in v
  - batched finale (reciprocal + broadcast multiply) per group, two
    output DMAs so group 0's store overlaps group 1's compute
"""

import sys
import types

import numpy as np
import ml_dtypes

B, S, D = 2, 4096, 512
HD = 64
SCALE = 128
SS = S // 4          # 1024 positions per shard
HP = SCALE           # halo padding each side
NP = SS + 2 * HP     # 1280 padded positions
NDC = D // 128       # 4 d-chunks
CA = 768             # region A cols (chunks 0-5, query blocks 0-3)
CB = NP - CA         # region B cols 768:1280 (chunks 6-9, blocks 4-7)

WARMUP = 6           # PE warmup matmuls under the DMA window (HAM un-throttle)

_CACHE = {}


def _ensure_hooks():
    """Register the axon NTFF profile hook; keep artifacts local."""
    if "antenv.axon_hooks" not in sys.modules:
        try:
            from trn_agent_boot.trn_boot import _ntff_profile_via_ctypes

            m = types.ModuleType("antenv.axon_hooks")
            m.get_axon_ntff_profile_hook = lambda: _ntff_profile_via_ctypes(
                "/opt/axon/libaxon_pjrt.so"
            )
            sys.modules["antenv.axon_hooks"] = m
        except Exception:
            pass
    import concourse.bass_utils as bass_utils

    bass_utils.upload_artifacts = lambda tmpdir: tmpdir


def _build_nc():
    import concourse.mybir as mybir
    import concourse.tile as tile
    from concourse import bacc

    bf = mybir.dt.bfloat16
    f32 = mybir.dt.float32
    AF = mybir.ActivationFunctionType
    AL = mybir.AluOpType

    nc = bacc.Bacc("TRN2", target_bir_lowering=False, debug=False, num_devices=8)

    xT_d = nc.dram_tensor("xT", [D, NP], bf, kind="ExternalInput")
    wm_d = nc.dram_tensor("wm", [128, 8, 128], bf, kind="ExternalInput")
    out_d = nc.dram_tensor("out", [128, 8, HD], bf, kind="ExternalOutput")

    with tile.TileContext(nc) as tc:
        with (
            tc.tile_pool(name="consts", bufs=1) as consts,
            tc.tile_pool(name="xtp", bufs=1) as xtp,
            tc.tile_pool(name="qkp", bufs=1) as qkp,
            tc.tile_pool(name="vgp", bufs=1) as vgp,
            tc.tile_pool(name="exp", bufs=2) as expool,
            tc.tile_pool(name="emp", bufs=2) as empool,
            tc.tile_pool(name="fin", bufs=1) as fin,
        ):
            wm_s = consts.tile([128, 8, 128], bf)
            xa = [xtp.tile([128, CA], bf, tag=f"xa{dc}", name=f"xa{dc}") for dc in range(NDC)]
            xb = [xtp.tile([128, CB], bf, tag=f"xb{dc}", name=f"xb{dc}") for dc in range(NDC)]

            # Input DMAs, byte-balanced across the two HWDGE engines.
            # sync: wm(256K) + xa1(192K) + xb1(128K) + xb3(128K) = 704K
            # scalar: xa0 + xa2 + xa3 (192K ea) + xb0 (128K) = 704K
            nc.sync.dma_start(out=wm_s, in_=wm_d[:, :, :])
            nc.scalar.dma_start(out=xa[0], in_=xT_d[0:128, 0:CA])
            nc.sync.dma_start(out=xa[1], in_=xT_d[128:256, 0:CA])
            nc.scalar.dma_start(out=xa[2], in_=xT_d[256:384, 0:CA])
            nc.scalar.dma_start(out=xa[3], in_=xT_d[384:512, 0:CA])
            nc.scalar.dma_start(out=xb[0], in_=xT_d[0:128, CA:NP])
            nc.sync.dma_start(out=xb[1], in_=xT_d[128:256, CA:NP])
            nc.scalar.dma_start(out=xb[2], in_=xT_d[256:384, CA:NP])
            nc.sync.dma_start(out=xb[3], in_=xT_d[384:512, CA:NP])

            # Trigger the ACT exp table load now (1.3us) so it hides
            # under the transfers.
            zz = consts.tile([128, 1], f32)
            nc.vector.memset(zz, 0.0)
            ez = consts.tile([128, 1], f32)
            nc.scalar.activation(ez, zz, AF.Exp)

            # DMA-free garbage tile for PE warmup.
            garb = consts.tile([128, 512], bf)
            nc.vector.memset(garb, 0.5)

            kTa = qkp.tile([64, CA], bf, tag="kTa")
            qTa = qkp.tile([64, 512], bf, tag="qTa")
            kTb = qkp.tile([64, CB], bf, tag="kTb")
            qTb = qkp.tile([64, 512], bf, tag="qTb")
            vaugA = vgp.tile([128, 6, 66], bf, tag="vaugA")
            nc.vector.memset(vaugA[:, :, 64:66], 1.0)
            vaugB = vgp.tile([128, 4, 66], bf, tag="vaugB")
            nc.vector.memset(vaugB[:, :, 64:66], 1.0)

            outsb = fin.tile([128, 8, HD], bf)
            rc = fin.tile([128, 8, 1], f32)

            def k_chunk(c):
                return kTa[:, c * 128 : (c + 1) * 128] if c < 6 else \
                    kTb[:, (c - 6) * 128 : (c - 5) * 128]

            def q_block(qb):
                return qTa[:, qb * 128 : (qb + 1) * 128] if qb < 4 else \
                    qTb[:, (qb - 4) * 128 : (qb - 3) * 128]

            def v_chunk(c):
                return vaugA[:, c, 0:65] if c < 6 else vaugB[:, c - 6, 0:65]

            with (
                tc.tile_pool(name="aps1", bufs=1, space="PSUM") as aps1,
            ):
                    qksB = aps1.tile([128, 512], f32, tag="qksB")
                    vpsB = aps1.tile([128, 4, HD], f32, tag="vpsB")
                    # PE warmup under the DMA window (HAM un-throttle).
                    wps = aps1.tile([128, 512], f32, tag="warm")
                    for _ in range(WARMUP):
                        nc.tensor.matmul(
                            wps, lhsT=garb[:, 0:128], rhs=garb, start=True, stop=True
                        )

                    qksA = aps1.tile([128, 768], f32, tag="qksA")
                    vpsA = aps1.tile([128, 6, HD], f32, tag="vpsA")

                    # Region A: packed q|k projection + v, d-chunk outer
                    # so matmuls start as each xa DMA lands.
                    for dc in range(NDC):
                        nc.tensor.matmul(
                            qksA[:, 0:512],
                            lhsT=wm_s[:, dc, :],
                            rhs=xa[dc][:, 0:512],
                            start=(dc == 0),
                            stop=(dc == NDC - 1),
                        )
                        nc.tensor.matmul(
                            qksA[:, 512:768],
                            lhsT=wm_s[:, dc, :],
                            rhs=xa[dc][:, 512:768],
                            start=(dc == 0),
                            stop=(dc == NDC - 1),
                        )
                    # v accumulation groups must be sequential per PSUM
                    # bank (start=True clears has_written for the WHOLE
                    # bank), so kc-outer / dc-inner.
                    for kc in range(6):
                        for dc in range(NDC):
                            nc.tensor.matmul(
                                vpsA[:, kc, :],
                                lhsT=xa[dc][:, kc * 128 : (kc + 1) * 128],
                                rhs=wm_s[:, 4 + dc // 2, (dc % 2) * 64 : (dc % 2) * 64 + 64],
                                start=(dc == 0),
                                stop=(dc == NDC - 1),
                            )
                    # Region B projections.
                    for dc in range(NDC):
                        nc.tensor.matmul(
                            qksB,
                            lhsT=wm_s[:, dc, :],
                            rhs=xb[dc],
                            start=(dc == 0),
                            stop=(dc == NDC - 1),
                        )
                    for kc in range(4):
                        for dc in range(NDC):
                            nc.tensor.matmul(
                                vpsB[:, kc, :],
                                lhsT=xb[dc][:, kc * 128 : (kc + 1) * 128],
                                rhs=wm_s[:, 4 + dc // 2, (dc % 2) * 64 : (dc % 2) * 64 + 64],
                                start=(dc == 0),
                                stop=(dc == NDC - 1),
                            )

                    # Region A evac: k and q read the same PSUM banks, so
                    # keep them on ONE engine (ACT); DVE handles v and the
                    # region-B evacs in parallel.
                    nc.scalar.copy(kTa, qksA[64:128, :])
                    nc.scalar.copy(qTa, qksA[0:64, 128:640])
                    nc.vector.tensor_copy(vaugA[:, :, 0:64], vpsA)
                    nc.vector.tensor_copy(qTb[:, 0:128], qksA[0:64, 640:768])
                    # Region B evac on DVE.
                    nc.vector.tensor_copy(kTb, qksB[64:128, :])
                    nc.vector.tensor_copy(qTb[:, 128:512], qksB[0:64, 0:384])
                    nc.vector.tensor_copy(vaugB[:, :, 0:64], vpsB)

            # All projection PSUM (6 banks) freed here; attention uses
            # sc (2x3 banks) + av (2x1 bank) = 8.
            with (
                tc.tile_pool(name="sps", bufs=2, space="PSUM") as sps,
                tc.tile_pool(name="avp", bufs=1, space="PSUM") as avp,
            ):
                    mask_b = wm_s[:, 6:8, :].unsqueeze(1).broadcast_to([128, 4, 2, 128])
                    scs, exs, ems = [], [], []
                    for g in range(2):
                        sc4 = sps.tile([128, 4, 3, 128], f32, tag="sc", name=f"sc{g}")
                        for b in range(4):
                            qb = g * 4 + b
                            for c in range(3):
                                nc.tensor.matmul(
                                    sc4[:, b, c, :],
                                    lhsT=k_chunk(qb + c),
                                    rhs=q_block(qb),
                                    start=True,
                                    stop=True,
                                )
                        ex4 = expool.tile([128, 4, 3, 128], bf, tag="ex", name=f"ex{g}")
                        nc.scalar.activation(ex4, sc4, AF.Exp, scale=0.125)
                        em4 = empool.tile([128, 4, 2, 128], bf, tag="em", name=f"em{g}")
                        nc.vector.tensor_tensor(
                            out=em4, in0=ex4[:, :, 0:3:2, :], in1=mask_b, op=AL.mult
                        )
                        scs.append(sc4)
                        exs.append(ex4)
                        ems.append(em4)

                    for g in range(2):
                        sc4, ex4, em4 = scs[g], exs[g], ems[g]
                        av = avp.tile([128, 4, 65], f32, tag=f"av{g}", name=f"av{g}")
                        for b in range(4):
                            qb = g * 4 + b
                            for c in range(3):
                                lhsT = (
                                    em4[:, b, 0, :]
                                    if c == 0
                                    else (ex4[:, b, 1, :] if c == 1 else em4[:, b, 1, :])
                                )
                                nc.tensor.matmul(
                                    av[:, b, 0:65],
                                    lhsT=lhsT,
                                    rhs=v_chunk(qb + c),
                                    start=(c == 0),
                                    stop=(c == 2),
                                )
                        sl = slice(g * 4, (g + 1) * 4)
                        nc.vector.reciprocal(rc[:, sl, :], av[:, :, 64:65])
                        nc.vector.tensor_tensor(
                            out=outsb[:, sl, :],
                            in0=av[:, :, 0:64],
                            in1=rc[:, sl, :].broadcast_to([128, 4, HD]),
                            op=AL.mult,
                        )
                        nc.sync.dma_start(
                            out=out_d[:, sl, :], in_=outsb[:, sl, :]
                        )

    nc.compile()
    return nc


def _get_nc():
    if "nc" not in _CACHE:
        _ensure_hooks()
        _CACHE["nc"] = _build_nc()
    return _CACHE["nc"]


def _host_inputs(inputs, wq, wk, wv):
    bf16 = ml_dtypes.bfloat16
    x = np.asarray(inputs, dtype=np.float32)
    wq = np.asarray(wq, dtype=np.float32)
    wk = np.asarray(wk, dtype=np.float32)
    wv = np.asarray(wv, dtype=np.float32)

    wm = np.zeros((128, 8, 128), np.float32)
    for dc in range(NDC):
        wm[:, dc, 0:64] = wq[dc * 128 : (dc + 1) * 128, :]
        wm[:, dc, 64:128] = wk[dc * 128 : (dc + 1) * 128, :]
    for dc in range(NDC):
        wm[:, 4 + dc // 2, (dc % 2) * 64 : (dc % 2) * 64 + 64] = (
            wv[dc * 128 : (dc + 1) * 128, :]
        )
    p = np.arange(128)[:, None]
    q = np.arange(128)[None, :]
    wm[:, 6, :] = (p >= q).astype(np.float32)
    wm[:, 7, :] = (p < q).astype(np.float32)
    wm = wm.astype(bf16)

    in_maps = []
    for i in range(8):
        b, c = divmod(i, 4)
        s0 = c * SS
        xp = np.zeros((NP, D), np.float32)
        lo = max(0, s0 - HP)
        hi = min(S, s0 + SS + HP)
        xp[lo - (s0 - HP) : hi - (s0 - HP)] = x[b, lo:hi]
        xT = np.ascontiguousarray(xp.T).astype(bf16)         # [512, 1280]
        in_maps.append({"xT": xT, "wm": wm})
    return in_maps


def run_sharded(inputs, wq, wk, wv, trace=False, trace_cores=None):
    """Run the SPMD kernel; returns (out [B,S,HD] f32, BassKernelResults)."""
    _ensure_hooks()
    import concourse.bass_utils as bass_utils

    nc = _get_nc()
    in_maps = _host_inputs(inputs, wq, wk, wv)
    res = bass_utils.run_bass_kernel_spmd(
        nc,
        in_maps,
        core_ids=list(range(8)),
        trace=trace,
        trace_cores=trace_cores,
    )
    out = np.empty((B, S, HD), np.float32)
    for i in range(8):
        b, c = divmod(i, 4)
        o = np.asarray(res.results[i]["out"]).astype(np.float32)  # [128, 8, 64]
        out[b, c * SS : (c + 1) * SS] = o.transpose(1, 0, 2).reshape(SS, HD)
    return out, res


def kernel(inputs, wq, wk, wv):
    out, _ = run_sharded(inputs, wq, wk, wv, trace=False)
    return out
